# revision 28
# baseline (speedup 1.0000x reference)
"""Multi-scale cross-attention Trainium2 kernel (8 NeuronCores, SPMD).

Reference computation (per batch b, scale s):
    q = x @ Wq                          [N, 512] -> heads [N, 8, 64]
    k, v = split(context @ Wkv)         [M, 512] each -> [M, 8, 64]
    sim_h = q_h @ k_h^T / 8             [N, M]  per head
    out_h = softmax(sim_h) @ v_h        [N, 64]
    out = concat_h(out_h) @ Wo + bo     [N, 512]

Shapes: B=2, N=1024, S=4, M=2048, H=8, D=64. Sharding: one (b, s) pair
per core (B*S == 8 cores); weights and context[b] are replicated, kv is
recomputed per core, so no collectives are needed.

Kernel-internal layout (per core):
    xT   [512, 1024]  (4x [128,1024] tiles)  - x_bs transposed via PE
    ctxT [512, 2048]  (4x [128,2048])        - context[b] transposed via PE
    qT   [512, 1024]  (4x [128,1024])        - q^T, head pair per tile
    kT   [512, 2048]  (4x [128,2048])        - k^T, head pair per tile
    v1   16x [128, 8*65]                     - v m-chunks, per head 64 cols + ones col
    simT = kT_h.T @ qT_h                     - scores transposed [m, n]; row-tiled
                                               K=64 matmul pairs (two heads concurrent)
    e^T  = exp(simT / 8)  (ACT, scale fused; scores are in [-8.5, 8.5] so no
                           max-subtraction is needed in f32)
    attn@v: psum[65, n] += v1_h.T @ e^T      - row 64 accumulates the softmax
                                               denominator (ones column trick)
    attT = psum[0:64] * bcast(1/psum[64])    - gpsimd partition_broadcast
    out  = attT.T @ Wo + bo                  - bias via K=1 ones-row matmul
"""

import os
from contextlib import ExitStack

import numpy as np

import concourse.bass as bass
import concourse.mybir as mybir
import concourse.tile as tile
from concourse import bacc
from concourse.bass import ds, ts
from concourse.bass_utils import run_bass_kernel_spmd
from concourse.masks import make_identity

F32 = mybir.dt.float32
BF16 = mybir.dt.bfloat16
F32R = mybir.dt.float32r
MM_MODE = os.environ.get("MM_MODE", "f32r")  # "f32r" | "bf16" | "f32"
SD = {"f32r": F32R, "bf16": BF16, "f32": F32}[MM_MODE]
LDT = F32R if MM_MODE == "f32r" else F32
AV_DT = F32 if MM_MODE == "f32" else BF16


def mm(ap):
    return ap
H = 8
D = 64
N = 1024
M = 2048
DIM = 512
N_CORES = 8


KDEBUG = os.environ.get("KDEBUG", "") == "1"


def emit(nc, x, ctxin, wq, wkv, wo, bo, out, dbg=None):
    """Emit the Tile program. All args are DRAM APs."""
    with tile.TileContext(nc) as tc, ExitStack() as top:
        Exp = mybir.ActivationFunctionType.Exp

        const = top.enter_context(tc.tile_pool(name="const", bufs=1))
        ident = const.tile([128, 128], F32, tag="ident")
        make_identity(nc, ident[:])
        ones_f = const.tile([128, 128], F32, tag="ones_f")
        nc.gpsimd.memset(ones_f[:], 1.0)
        ones_row = const.tile([1, 128], SD, tag="ones_row")
        nc.vector.tensor_copy(ones_row[:], ones_f[0:1, :])
        ones8 = const.tile([128, 8], SD, tag="ones8")
        nc.vector.tensor_copy(ones8[:], ones_f[:, 0:8])
        identr = const.tile([128, 128], LDT, tag="identr")
        nc.vector.tensor_copy(identr[:], ident[:])
        bo_sb = const.tile([1, DIM], SD, tag="bo")

        # Single psum pool: tags A and B, [128,1024] slots, 2 bufs each ->
        # 8 banks; phase-1 groups alternate A/B, attention uses A for scores
        # and B for the attn@v accumulators.
        psum = top.enter_context(tc.tile_pool(name="psum", bufs=2, space="PSUM"))
        _psn = [0]
        _evn = [0]

        def psalt(shape, dtype, name):
            _psn[0] += 1
            return psum.tile(shape, dtype, tag="AB"[_psn[0] % 2], name=name)

        psA = lambda shape, dtype, name: psum.tile(shape, dtype, tag="A", name=name)
        psB = lambda shape, dtype, name: psum.tile(shape, dtype, tag="B", name=name)

        # phase-1 psum evacuations alternate between ScalarE (idle until
        # attention) and VectorE so neither engine serializes phase 1
        def evac(dst, src_):
            _evn[0] += 1
            if _evn[0] % 2 == 0:
                nc.scalar.copy(dst, src_)
            else:
                nc.vector.tensor_copy(dst, src_)

        persist = top.enter_context(tc.tile_pool(name="persist", bufs=1))
        qT = [persist.tile([128, N], SD, tag=f"qT{i}", name=f"qT{i}") for i in range(4)]
        kT = [persist.tile([128, M], SD, tag=f"kT{i}", name=f"kT{i}") for i in range(4)]
        v1 = [persist.tile([128, H * (D + 1)], SD, tag=f"v1_{i}", name=f"v1_{i}")
              for i in range(16)]
        wo_sb = [persist.tile([128, DIM], SD, tag=f"wo{i}", name=f"wo{i}") for i in range(4)]

        # ---------------- phase 1: transposes + projections ----------------
        with ExitStack() as ph1:
            ld = ph1.enter_context(tc.tile_pool(name="ld", bufs=10))
            wpool = ph1.enter_context(tc.tile_pool(name="wpool", bufs=1))
            tpool = ph1.enter_context(tc.tile_pool(name="tpool", bufs=1))
            wq_sb = [wpool.tile([128, DIM], SD, tag=f"wq{i}", name=f"wq{i}") for i in range(4)]
            wkv_sb = [wpool.tile([128, 2 * DIM], SD, tag=f"wkv{i}", name=f"wkv{i}")
                      for i in range(4)]
            xT = [tpool.tile([128, N], SD, tag=f"xT{i}", name=f"xT{i}") for i in range(4)]
            ctxT = [tpool.tile([128, M], SD, tag=f"ctxT{i}", name=f"ctxT{i}") for i in range(4)]
            wdma = nc.gpsimd.dma_start if MM_MODE == "bf16" else nc.sync.dma_start

            def transposes(src, dstT, rg):
                ldt = [ld.tile([128, DIM], LDT, tag="ld", name="ldt") for _ in range(8)]
                for j in range(8):
                    nc.sync.dma_start(out=ldt[j][:], in_=src[ts(rg * 8 + j, 128), :])
                for ic in range(4):
                    tp = psA([128, 1024], LDT, "tp")
                    for j in range(8):
                        nc.tensor.transpose(
                            tp[:, ts(j, 128)], ldt[j][:, ts(ic, 128)], identr[:]
                        )
                    evac(dstT[ic][:, ts(rg, 1024)], tp[:])

            def emit_qT(oc):
                pq = psA([128, 1024], F32, "pq")
                for ic in range(4):
                    for nw in range(2):
                        nc.tensor.matmul(
                            pq[:, ts(nw, 512)],
                            lhsT=wq_sb[ic][:, ts(oc, 128)],
                            rhs=xT[ic][:, ts(nw, 512)],
                            start=(ic == 0),
                            stop=(ic == 3),
                        )
                evac(qT[oc][:], pq[:])

            def emit_kT(oc):
                for mw in range(2):
                    pk = psA([128, 1024], F32, "pk")
                    for ic in range(4):
                        for nw in range(2):
                            nc.tensor.matmul(
                                pk[:, ts(nw, 512)],
                                lhsT=wkv_sb[ic][:, ts(oc, 128)],
                                rhs=ctxT[ic][:, ds(mw * 1024 + nw * 512, 512)],
                                start=(ic == 0),
                                stop=(ic == 3),
                            )
                    evac(kT[oc][:, ts(mw, 1024)], pk[:])

            def emit_v1(mc):
                # v1[mc] cols h*65..h*65+63 = v m-chunk mc head h; +64 = ones
                pv = psA([128, 512], F32, "pv")
                for ic in range(4):
                    nc.tensor.matmul(
                        pv[:],
                        lhsT=ctxT[ic][:, ts(mc, 128)],
                        rhs=wkv_sb[ic][:, ds(DIM, DIM)],
                        start=(ic == 0),
                        stop=(ic == 3),
                    )
                nc.vector.tensor_copy(
                    v1[mc][:].rearrange("p (h j) -> p h j", j=D + 1)[:, :, 0:D],
                    pv[:].rearrange("p (h j) -> p h j", j=D),
                )
                nc.vector.tensor_copy(
                    v1[mc][:].rearrange("p (h j) -> p h j", j=D + 1)[:, :, D],
                    ones8[:],
                )

            transposes(x, xT, 0)
            for i in range(4):
                wdma(out=wq_sb[i][:], in_=wq[ts(i, 128), :])
            for rg in range(2):
                transposes(ctxin, ctxT, rg)
            for i in range(4):
                wdma(out=wkv_sb[i][:], in_=wkv[ts(i, 128), :])
                wdma(out=wo_sb[i][:], in_=wo[ts(i, 128), :])
            wdma(out=bo_sb[:], in_=bo[:])
            emit_qT(0)
            emit_kT(0)
            for oc in range(1, 4):
                emit_qT(oc)
                emit_kT(oc)
            for mc in range(16):
                emit_v1(mc)

        late = top.enter_context(tc.tile_pool(name="late", bufs=1))
        attT = [late.tile([128, N], SD, tag=f"attT{i}", name=f"attT{i}") for i in range(4)]
        e_pool = top.enter_context(tc.tile_pool(name="e_pool", bufs=8))
        nrm = top.enter_context(tc.tile_pool(name="nrm", bufs=2))

        # ---------------- phase 2: attention ----------------
        for p in range(4):  # head pair: heads 2p (base 0), 2p+1 (base 64)
            po = [psB([D + 1, N], F32, "po") for _ in range(2)]
            prev = None

            def emit_av(pet, pmc, last):
                for hh in range(2):
                    for nw in range(2):
                        nc.tensor.matmul(
                            po[hh][:, ts(nw, 512)],
                            lhsT=v1[pmc][:, ds((2 * p + hh) * (D + 1), D + 1)],
                            rhs=pet[hh][:, ts(nw, 512)],
                            start=(pmc == 0),
                            stop=last,
                        )

            for mc in range(16):
                if prev is not None:
                    emit_av(*prev, False)
                et = []
                for hh in range(2):
                    bp = 64 * hh
                    ps = psA([128, N], F32, "ps")
                    # K=64; base partition 0/64 row-tiles the two heads onto
                    # disjoint PE row-groups (concurrent)
                    for nw in range(2):
                        nc.tensor.matmul(
                            ps[:, ts(nw, 512)],
                            lhsT=kT[p][ds(bp, 64), ts(mc, 128)],
                            rhs=qT[p][ds(bp, 64), ts(nw, 512)],
                            start=True,
                            stop=True,
                        )
                    e = e_pool.tile([128, N], SD, tag="e", name="et")
                    nc.scalar.activation(e[:], ps[:], Exp, scale=1.0 / 8.0)
                    if dbg is not None and p == 0 and mc == 0 and hh == 0:
                        nc.sync.dma_start(out=dbg["e00"][:], in_=e[:])
                    et.append(e)
                prev = (et, mc)
            emit_av(*prev, True)

            # normalize: attT[h] = po[0:64] * bcast(1 / po[64])
            for hh in range(2):
                bp = 64 * hh
                poe = nrm.tile([D + 1, N], F32, tag="poe", name="poe")
                nc.vector.tensor_copy(poe[:], po[hh][:])
                # reciprocal is ~8 cyc/elem serial per lane: spread the
                # row-sums over 128 partitions via SBUF DMA first
                rsp = nrm.tile([128, N // 128], F32, tag="rsp", name="rsp")
                nc.sync.dma_start(out=rsp[:], in_=poe[ds(D, 1), :])
                nc.vector.reciprocal(rsp[:], rsp[:])
                rec = nrm.tile([1, N], F32, tag="rec", name="rec")
                nc.sync.dma_start(out=rec[:], in_=rsp[:])
                if dbg is not None and p == 0 and hh == 0:
                    nc.sync.dma_start(out=dbg["po0"][:], in_=poe[:])
                    nc.sync.dma_start(out=dbg["rec0"][:], in_=rec[:])
                bc = nrm.tile([D, N], F32, tag="poe", name="bc")
                nc.gpsimd.partition_broadcast(bc[:], rec[:])
                nc.vector.tensor_mul(attT[p][ds(bp, D), :], poe[0:D, :], bc[:])

        if dbg is not None:
            nc.sync.dma_start(out=dbg["qt0"][:], in_=qT[0][:])
            nc.sync.dma_start(out=dbg["kt0"][:], in_=kT[0][:])
            nc.sync.dma_start(out=dbg["v0"][:], in_=v1[0][:])
            nc.sync.dma_start(out=dbg["att0"][:], in_=attT[0][:])

        # ---------------- phase 3: output projection ----------------
        with ExitStack() as ph3:
            o_sb = ph3.enter_context(tc.tile_pool(name="o_sb", bufs=3))
            for r in range(8):
                pout = psA([128, DIM], F32, "pout")
                for ic in range(4):
                    nc.tensor.matmul(
                        pout[:],
                        lhsT=attT[ic][:, ts(r, 128)],
                        rhs=wo_sb[ic][:],
                        start=(ic == 0),
                        stop=False,
                    )
                nc.tensor.matmul(
                    pout[:],
                    lhsT=ones_row[:],
                    rhs=bo_sb[:],
                    start=False,
                    stop=True,
                )
                ot = o_sb.tile([128, DIM], F32, tag="ot")
                nc.vector.tensor_copy(ot[:], pout[:])
                nc.sync.dma_start(out=out[ts(r, 128), :], in_=ot[:])


_PROGRAM = None


def build_program():
    global _PROGRAM
    if _PROGRAM is not None:
        return _PROGRAM
    nc = bacc.Bacc("TRN2", target_bir_lowering=False, debug=False, num_devices=N_CORES)
    x = nc.declare_dram_parameter("x", [N, DIM], LDT, isOutput=False)
    ctxin = nc.declare_dram_parameter("ctx", [M, DIM], LDT, isOutput=False)
    wdt = F32 if MM_MODE == "bf16" else SD
    wq = nc.declare_dram_parameter("wq", [DIM, DIM], wdt, isOutput=False)
    wkv = nc.declare_dram_parameter("wkv", [DIM, 2 * DIM], wdt, isOutput=False)
    wo = nc.declare_dram_parameter("wo", [DIM, DIM], wdt, isOutput=False)
    bo = nc.declare_dram_parameter("bo", [1, DIM], wdt, isOutput=False)
    out = nc.declare_dram_parameter("out", [N, DIM], F32, isOutput=True)
    dbg = None
    if KDEBUG:
        dbg = {
            "qt0": nc.declare_dram_parameter("qt0", [128, N], SD, isOutput=True),
            "kt0": nc.declare_dram_parameter("kt0", [128, M], SD, isOutput=True),
            "v0": nc.declare_dram_parameter("v0", [128, H * (D + 1)], SD, isOutput=True),
            "e00": nc.declare_dram_parameter("e00", [128, N], SD, isOutput=True),
            "po0": nc.declare_dram_parameter("po0", [D + 1, N], F32, isOutput=True),
            "rec0": nc.declare_dram_parameter("rec0", [1, N], F32, isOutput=True),
            "att0": nc.declare_dram_parameter("att0", [128, N], SD, isOutput=True),
        }
    emit(nc, x, ctxin, wq, wkv, wo, bo, out, dbg)
    nc.compile()
    _PROGRAM = nc
    return nc


def make_in_maps(x, context, Wq, Wkv, Wo, bo):
    """Shard host-side: core c -> (b, s) = (c // 4, c % 4)."""
    x = np.ascontiguousarray(x, dtype=np.float32)
    context = np.ascontiguousarray(context, dtype=np.float32)
    in_maps = []
    for c in range(N_CORES):
        b, s = c // 4, c % 4
        in_maps.append(
            {
                "x": np.ascontiguousarray(x[b, :, s, :]),
                "ctx": context[b],
                "wq": np.ascontiguousarray(Wq, dtype=np.float32),
                "wkv": np.ascontiguousarray(Wkv, dtype=np.float32),
                "wo": np.ascontiguousarray(Wo, dtype=np.float32),
                "bo": np.ascontiguousarray(bo, dtype=np.float32).reshape(1, DIM),
            }
        )
    return in_maps


def run(inputs, trace=False, **spmd_kwargs):
    nc = build_program()
    in_maps = make_in_maps(
        inputs["x"], inputs["context"], inputs["Wq"], inputs["Wkv"],
        inputs["Wo"], inputs["bo"],
    )
    res = run_bass_kernel_spmd(
        nc, in_maps, list(range(N_CORES)), trace=trace, **spmd_kwargs
    )
    B, S = 2, 4
    out = np.empty((B, N, S, DIM), dtype=np.float32)
    for c in range(N_CORES):
        b, s = c // 4, c % 4
        out[b, :, s, :] = res.results[c]["out"]
    return out, res


def kernel(**inputs):
    out, _ = run(inputs, trace=False)
    return out
